# revision 1
# baseline (speedup 1.0000x reference)
"""MoE routing kernel for Trainium2, expert-parallel across 8 NeuronCores.

Sharding: core c owns experts [8c, 8c+8). The gate/top-k/dispatch-position
computation runs on host as part of the sharding step; each core receives its
experts' dispatched token rows (transposed, bf16), its expert weights, and a
slice of tokens for the (replicated-weight) shared expert. Device computes the
grouped SwiGLU expert GEMMs + shared expert. Host gathers per-slot outputs and
does the weighted combine (unshard).
"""

import os

import numpy as np
import ml_dtypes

import bass_rust
import concourse.bass as bass
import concourse.mybir as mybir
from concourse.tile import TileContext
from concourse.vector_clock import ScopedClock
from concourse.bass_utils import run_bass_kernel_spmd

B, T, C = 2, 2048, 2048
N = B * T
E, H, HS = 64, 256, 512
TOPK = 6
NCORES = 8
ELOC = E // NCORES  # 8 experts per core
NLOC = N // NCORES  # 512 tokens per core for the shared expert
BF16 = mybir.dt.bfloat16
F32 = mybir.dt.float32
P = 128

_BF16_NP = ml_dtypes.bfloat16


# --------------------------------------------------------------------------
# Tile tail-drain fix: this walrus build allows at most one semaphore wait per
# instruction (none on Drain). Tile's end-of-context drain carries the whole
# global clock; emit a chain of single-wait NOPs on SP instead.
# --------------------------------------------------------------------------
def _patched_drain_and_barrier(self, tick_clock, wait_clock):
    carrier = self.nc.sync.nop(nofuse=True, hint="tail_wait_0")
    wait_clock.add_sem_waits(carrier.ins, ScopedClock({None: tick_clock.global_clock}))
    si = carrier.ins.sync_info
    waits = list(si.on_wait) if si else []
    upds = list(si.on_update) if si else []
    carrier.ins.sync_info = bass_rust.SyncInfo(on_wait=waits[:1], on_update=upds)
    for i, w in enumerate(waits[1:]):
        n2 = self.nc.sync.nop(nofuse=True, hint=f"tail_wait_{i + 1}")
        n2.ins.sync_info = bass_rust.SyncInfo(on_wait=[w], on_update=[])

    self.nc.sync.drain()
    self.nc.all_engine_barrier()
    assert self.sems is not None
    popped = self.nc._tile_sem_poison_stack.pop()
    assert popped is self._sem_poison
    self.nc.clear_and_free_semaphores(list(self.sems.allocated().values()))
    self.nc.all_engine_barrier()


_orig_add_instruction = TileContext._add_instruction


def _patched_add_instruction(self, inst):
    si = getattr(inst, "sync_info", None)
    if si is not None and len(si.on_wait) > 1:
        waits = list(si.on_wait)
        for w in waits[:-1]:
            nop = mybir.InstNoOp(
                name=self.nc.get_next_instruction_name(), ins=[], outs=[])
            nop.engine = inst.engine
            nop.sync_info = bass_rust.SyncInfo(on_wait=[w], on_update=[])
            _orig_add_instruction(self, nop)
        inst.sync_info = bass_rust.SyncInfo(
            on_wait=[waits[-1]], on_update=list(si.on_update))
    _orig_add_instruction(self, inst)


def _install_drain_fix():
    if getattr(TileContext, "_drain_fix_installed", False):
        return
    TileContext._drain_and_barrier = _patched_drain_and_barrier
    TileContext._add_instruction = _patched_add_instruction
    TileContext._drain_fix_installed = True


# --------------------------------------------------------------------------
# Device kernel
# --------------------------------------------------------------------------
_BUILD_CACHE = {}


def _build(cap):
    """Build the per-core Bass program; cap = padded per-expert capacity."""
    _install_drain_fix()
    nc = bass.Bass()

    xdT = nc.declare_dram_parameter("xdT", [ELOC, C, cap], BF16, isOutput=False)
    wup = nc.declare_dram_parameter("wup", [ELOC, C, 2 * H], BF16, isOutput=False)
    wdn = nc.declare_dram_parameter("wdn", [ELOC, H, C], BF16, isOutput=False)
    xsT = nc.declare_dram_parameter("xsT", [C, NLOC], BF16, isOutput=False)
    wsu = nc.declare_dram_parameter("wsu", [C, 2 * HS], BF16, isOutput=False)
    wsd = nc.declare_dram_parameter("wsd", [HS, C], BF16, isOutput=False)
    yr = nc.declare_dram_parameter("yr", [ELOC * cap, C], BF16, isOutput=True)
    ysh = nc.declare_dram_parameter("ysh", [NLOC, C], BF16, isOutput=True)

    KC = C // P          # 16 contraction chunks over C
    MU = (2 * H) // P    # 4 output chunks of up-proj (2H = 512)
    KH = H // P          # 2 contraction chunks over H
    NCC = C // 512       # 4 output column chunks of down-proj
    SC = cap // P        # slot chunks per expert
    assert cap % P == 0

    with TileContext(nc) as tc:
        with (
            tc.tile_pool(name="wu_sb", bufs=24) as wu_pool,
            tc.tile_pool(name="xd_sb", bufs=24) as xd_pool,
            tc.tile_pool(name="wd_sb", bufs=4) as wd_pool,
            tc.tile_pool(name="h_sb", bufs=8) as h_pool,
            tc.tile_pool(name="sg_sb", bufs=4) as sg_pool,
            tc.tile_pool(name="o_sb", bufs=6) as o_pool,
            tc.tile_pool(name="sh_sb", bufs=KC) as sh_pool,
            tc.tile_pool(name="pu", bufs=6, space="PSUM") as pu_pool,
            tc.tile_pool(name="pd", bufs=2, space="PSUM") as pd_pool,
        ):
            # ---------------- shared expert (512 local tokens) -------------
            xs_tiles = []
            for k in range(KC):
                t = sh_pool.tile([P, NLOC], BF16, tag="xs")
                nc.sync.dma_start(out=t[:], in_=xsT[k * P:(k + 1) * P, :])
                xs_tiles.append(t)

            wsu_tiles = []
            for k in range(KC):
                t = sh_pool.tile([P, 2 * HS], BF16, tag="wsu")
                nc.sync.dma_start(out=t[:], in_=wsu[k * P:(k + 1) * P, :])
                wsu_tiles.append(t)

            hsh_tiles = []  # [HS part chunks (4), NLOC] bf16, h = silu(g_s)*y_s
            for half in range(2):  # process 2H_S=1024 in halves of 512 cols
                ps_tiles = []
                for m in range(4):
                    mm = half * 4 + m
                    pt = pu_pool.tile([P, NLOC], F32, space="PSUM", tag="pu")
                    for k in range(KC):
                        nc.tensor.matmul(
                            out=pt[:],
                            lhsT=wsu_tiles[k][:, mm * P:(mm + 1) * P],
                            rhs=xs_tiles[k][:],
                            start=(k == 0), stop=(k == KC - 1))
                    ps_tiles.append(pt)
                if half == 0:
                    # channels 0:512 = y_s (chunk order: y first); move out of
                    # PSUM so the second half can reuse the banks
                    y_s_tiles = []
                    for j in range(4):
                        yt = sg_pool.tile([P, NLOC], F32, tag="ys")
                        nc.vector.tensor_copy(out=yt[:], in_=ps_tiles[j][:])
                        y_s_tiles.append(yt)
                else:
                    # channels 512:1024 = g_s; h = silu(g_s) * y_s
                    for j in range(4):
                        sg = sg_pool.tile([P, NLOC], F32, tag="sg")
                        nc.scalar.activation(sg[:], ps_tiles[j][:],
                                             mybir.ActivationFunctionType.Silu)
                        ht = h_pool.tile([P, NLOC], BF16, tag="h")
                        nc.vector.tensor_mul(ht[:], sg[:], y_s_tiles[j][:])
                        hsh_tiles.append(ht)

            wsd_tiles = []
            for k in range(4):  # HS = 512 -> 4 chunks
                t = wd_pool.tile([P, C], BF16, tag="wsd")
                nc.sync.dma_start(out=t[:], in_=wsd[k * P:(k + 1) * P, :])
                wsd_tiles.append(t)
            for mt in range(NLOC // P):  # 4 token chunks
                for ncc in range(NCC):
                    pt = pd_pool.tile([P, 512], F32, space="PSUM", tag="pd")
                    for k in range(4):
                        nc.tensor.matmul(
                            out=pt[:],
                            lhsT=hsh_tiles[k][:, mt * P:(mt + 1) * P],
                            rhs=wsd_tiles[k][:, ncc * 512:(ncc + 1) * 512],
                            start=(k == 0), stop=(k == 3))
                    ot = o_pool.tile([P, 512], BF16, tag="osh")
                    nc.vector.tensor_copy(out=ot[:], in_=pt[:])
                    nc.scalar.dma_start(
                        out=ysh[mt * P:(mt + 1) * P, ncc * 512:(ncc + 1) * 512],
                        in_=ot[:])

            # ---------------- routed experts ------------------------------
            for e in range(ELOC):
                # up-projection: psum[m] = [128 of 2H, cap slots]
                xd_tiles = []
                for k in range(KC):
                    t = xd_pool.tile([P, cap], BF16, tag="xd")
                    nc.sync.dma_start(
                        out=t[:], in_=xdT[e, k * P:(k + 1) * P, :])
                    xd_tiles.append(t)
                wu_tiles = []
                for k in range(KC):
                    wt = wu_pool.tile([P, 2 * H], BF16, tag="wu")
                    nc.sync.dma_start(out=wt[:], in_=wup[e, k * P:(k + 1) * P, :])
                    wu_tiles.append(wt)
                up_tiles = []
                for m in range(MU):
                    pt = pu_pool.tile([P, cap], F32, space="PSUM", tag="pu")
                    for k in range(KC):
                        nc.tensor.matmul(
                            out=pt[:],
                            lhsT=wu_tiles[k][:, m * P:(m + 1) * P],
                            rhs=xd_tiles[k][:],
                            start=(k == 0), stop=(k == KC - 1))
                    up_tiles.append(pt)
                # g = chunks 0..1 (first 256 channels), v = chunks 2..3
                h_tiles = []
                for j in range(KH):
                    sg = sg_pool.tile([P, cap], F32, tag="sg2")
                    nc.scalar.activation(sg[:], up_tiles[j][:],
                                         mybir.ActivationFunctionType.Silu)
                    ht = h_pool.tile([P, cap], BF16, tag="h2")
                    nc.vector.tensor_mul(ht[:], sg[:], up_tiles[KH + j][:])
                    h_tiles.append(ht)
                # down-projection: lhsT = h slot-chunk, rhs = w_down columns
                wd_tiles = []
                for k in range(KH):
                    t = wd_pool.tile([P, C], BF16, tag="wd")
                    nc.sync.dma_start(
                        out=t[:], in_=wdn[e, k * P:(k + 1) * P, :])
                    wd_tiles.append(t)
                for ms in range(SC):
                    for ncc in range(NCC):
                        pt = pd_pool.tile([P, 512], F32, space="PSUM", tag="pd")
                        for k in range(KH):
                            nc.tensor.matmul(
                                out=pt[:],
                                lhsT=h_tiles[k][:, ms * P:(ms + 1) * P],
                                rhs=wd_tiles[k][:, ncc * 512:(ncc + 1) * 512],
                                start=(k == 0), stop=(k == KH - 1))
                        ot = o_pool.tile([P, 512], BF16, tag="ord")
                        nc.vector.tensor_copy(out=ot[:], in_=pt[:])
                        row0 = e * cap + ms * P
                        nc.scalar.dma_start(
                            out=yr[row0:row0 + P, ncc * 512:(ncc + 1) * 512],
                            in_=ot[:])
    return nc


# --------------------------------------------------------------------------
# Host wrapper
# --------------------------------------------------------------------------
def kernel(x, w_gate, w_shared_up, w_shared_down, w_up, w_down):
    x_flat = x.reshape(-1, C)

    # ---- gate: sigmoid scores, top-6, normalized weights (f64 for a stable
    # ordering; ties in the fp32 reference are measure-zero) ----
    logits = x_flat.astype(np.float64) @ w_gate.astype(np.float64)
    scores = 1.0 / (1.0 + np.exp(-logits))
    topk_idx = np.argsort(-scores, axis=-1, kind="stable")[:, :TOPK]
    w = np.take_along_axis(scores, topk_idx, axis=-1)
    w = w / w.sum(-1, keepdims=True)

    # ---- dispatch positions (stable within each expert, slot-major order) --
    flat_e = topk_idx.reshape(-1)
    order = np.argsort(flat_e, kind="stable")
    sorted_e = flat_e[order]
    group_start = np.searchsorted(sorted_e, np.arange(E))
    pos = np.empty(N * TOPK, dtype=np.int64)
    pos[order] = np.arange(N * TOPK) - group_start[sorted_e]
    counts = np.bincount(flat_e, minlength=E)

    cap = 512
    mx = int(counts.max())
    if mx > cap:
        cap = ((mx + P - 1) // P) * P

    # ---- build per-core inputs ----
    xT_bf = np.ascontiguousarray(x_flat.T).astype(_BF16_NP)  # [C, N]
    wup_bf = w_up.astype(_BF16_NP)
    wdn_bf = w_down.astype(_BF16_NP)
    wsu_bf = w_shared_up.astype(_BF16_NP)
    wsd_bf = w_shared_down.astype(_BF16_NP)

    token_of_slot = np.arange(N * TOPK) // TOPK
    in_maps = []
    expert_tokens = []
    for e in range(E):
        slots = order[group_start[e]: group_start[e] + counts[e]]
        expert_tokens.append(token_of_slot[slots])
    for c in range(NCORES):
        xdT = np.zeros((ELOC, C, cap), dtype=_BF16_NP)
        for j in range(ELOC):
            tok = expert_tokens[c * ELOC + j]
            xdT[j][:, : len(tok)] = xT_bf[:, tok]
        xsT = np.ascontiguousarray(xT_bf[:, c * NLOC:(c + 1) * NLOC])
        in_maps.append({
            "xdT": xdT,
            "wup": wup_bf[c * ELOC:(c + 1) * ELOC],
            "wdn": wdn_bf[c * ELOC:(c + 1) * ELOC],
            "xsT": xsT,
            "wsu": wsu_bf,
            "wsd": wsd_bf,
        })

    if cap not in _BUILD_CACHE:
        _BUILD_CACHE[cap] = _build(cap)
    nc = _BUILD_CACHE[cap]

    res = run_bass_kernel_spmd(nc, in_maps, list(range(NCORES)))
    if res.exec_time_ns is not None:
        print(f"HW exec time: {res.exec_time_ns} ns", flush=True)

    # ---- host combine (unshard): gather per-slot rows, weight, sum ----
    yr_all = np.concatenate(
        [r["yr"].reshape(ELOC, cap, C) for r in res.results], axis=0)
    y_ts = yr_all[flat_e, pos].astype(np.float32)          # [N*K, C]
    routed = (y_ts.reshape(N, TOPK, C)
              * w.reshape(N, TOPK, 1).astype(np.float32)).sum(axis=1)
    shared = np.concatenate([r["ysh"] for r in res.results], axis=0).astype(np.float32)
    return (shared + routed).reshape(B, T, C).astype(np.float32)



# revision 2
# speedup vs baseline: 1.1906x; 1.1906x over previous
"""MoE routing kernel for Trainium2, expert-parallel across 8 NeuronCores.

Sharding: experts are snake-dealt to cores by descending token count so every
core sees the same per-position capacity profile (single SPMD program).  The
gate/top-k/dispatch runs on host as part of the sharding step; each core
receives its experts' dispatched token columns packed contiguously (exact
counts rounded to 8 — no fixed-capacity padding), its expert weights, and a
token slice for the replicated shared expert.  The device computes the grouped
SwiGLU expert GEMMs (down-projection transposed so cost scales with the exact
slot count) plus the shared expert.  Host gathers per-slot outputs and does
the weighted combine.
"""

import numpy as np
import ml_dtypes

import bass_rust
import concourse.bass as bass
import concourse.mybir as mybir
from concourse.tile import TileContext
from concourse.vector_clock import ScopedClock
from concourse.bass_utils import run_bass_kernel_spmd

B, T, C = 2, 2048, 2048
N = B * T
E, H, HS = 64, 256, 512
TOPK = 6
CAP = 1024
NCORES = 8
ELOC = E // NCORES   # 8 experts per core
NLOC = N // NCORES   # 512 tokens per core for the shared expert
BF16 = mybir.dt.bfloat16
F32 = mybir.dt.float32
P = 128
KC = C // P          # 16 contraction chunks over C

_BF16_NP = ml_dtypes.bfloat16


# --------------------------------------------------------------------------
# Tile tail-drain fix: this walrus build allows at most one semaphore wait per
# instruction (none on Drain). Tile's end-of-context drain carries the whole
# global clock; emit a chain of single-wait NOPs on SP instead.
# --------------------------------------------------------------------------
def _patched_drain_and_barrier(self, tick_clock, wait_clock):
    carrier = self.nc.sync.nop(nofuse=True, hint="tail_wait_0")
    wait_clock.add_sem_waits(carrier.ins, ScopedClock({None: tick_clock.global_clock}))
    si = carrier.ins.sync_info
    waits = list(si.on_wait) if si else []
    upds = list(si.on_update) if si else []
    carrier.ins.sync_info = bass_rust.SyncInfo(on_wait=waits[:1], on_update=upds)
    for i, w in enumerate(waits[1:]):
        n2 = self.nc.sync.nop(nofuse=True, hint=f"tail_wait_{i + 1}")
        n2.ins.sync_info = bass_rust.SyncInfo(on_wait=[w], on_update=[])

    self.nc.sync.drain()
    self.nc.all_engine_barrier()
    assert self.sems is not None
    popped = self.nc._tile_sem_poison_stack.pop()
    assert popped is self._sem_poison
    self.nc.clear_and_free_semaphores(list(self.sems.allocated().values()))
    self.nc.all_engine_barrier()


_orig_add_instruction = TileContext._add_instruction


def _patched_add_instruction(self, inst):
    si = getattr(inst, "sync_info", None)
    if si is not None and len(si.on_wait) > 1:
        waits = list(si.on_wait)
        for w in waits[:-1]:
            nop = mybir.InstNoOp(
                name=self.nc.get_next_instruction_name(), ins=[], outs=[])
            nop.engine = inst.engine
            nop.sync_info = bass_rust.SyncInfo(on_wait=[w], on_update=[])
            _orig_add_instruction(self, nop)
        inst.sync_info = bass_rust.SyncInfo(
            on_wait=[waits[-1]], on_update=list(si.on_update))
    _orig_add_instruction(self, inst)


def _install_drain_fix():
    if getattr(TileContext, "_drain_fix_installed", False):
        return
    TileContext._drain_and_barrier = _patched_drain_and_barrier
    TileContext._add_instruction = _patched_add_instruction
    TileContext._drain_fix_installed = True


# --------------------------------------------------------------------------
# Device kernel
# --------------------------------------------------------------------------
_BUILD_CACHE = {}


def _build(caps):
    """Per-core Bass program; caps = per-position slot capacities (all cores
    share the profile, multiples of 8)."""
    _install_drain_fix()
    nc = bass.Bass()
    S_tot = sum(caps)
    Smax = max(caps)
    offs = np.concatenate([[0], np.cumsum(caps)]).astype(int)

    # p-major packed inputs: element [p, k, j] = full[k*128 + p, j]
    xdp = nc.declare_dram_parameter("xdp", [P, KC, S_tot], BF16, isOutput=False)
    wup = nc.declare_dram_parameter("wup", [P, KC, ELOC * 512], BF16, isOutput=False)
    wdn = nc.declare_dram_parameter("wdn", [P, 2, ELOC * 2048], BF16, isOutput=False)
    xsp = nc.declare_dram_parameter("xsp", [P, KC, NLOC], BF16, isOutput=False)
    wsu = nc.declare_dram_parameter("wsu", [P, KC, 1024], BF16, isOutput=False)
    wsd = nc.declare_dram_parameter("wsd", [P, 4, 2048], BF16, isOutput=False)
    # transposed outputs: [C row, slot/token col]
    yrT = nc.declare_dram_parameter("yrT", [C, S_tot], BF16, isOutput=True)
    yshT = nc.declare_dram_parameter("yshT", [C, NLOC], BF16, isOutput=True)

    with TileContext(nc) as tc:
        with (
            tc.tile_pool(name="sh_sb", bufs=1) as sh_pool,
            tc.tile_pool(name="xd_sb", bufs=3) as xd_pool,
            tc.tile_pool(name="wu_sb", bufs=2) as wu_pool,
            tc.tile_pool(name="wd_sb", bufs=3) as wd_pool,
            tc.tile_pool(name="sg_sb", bufs=4) as sg_pool,
            tc.tile_pool(name="h_sb", bufs=4) as h_pool,
            tc.tile_pool(name="o_sb", bufs=8) as o_pool,
            tc.tile_pool(name="pu", bufs=4, space="PSUM") as pu_pool,
            tc.tile_pool(name="pd", bufs=4, space="PSUM") as pd_pool,
        ):
            # ---- shared-expert input loads (sync queue) -------------------
            xs_t = sh_pool.tile([P, KC, NLOC], BF16, tag="xs")
            nc.sync.dma_start(out=xs_t[:], in_=xsp[:])
            wsu_t = sh_pool.tile([P, KC, 1024], BF16, tag="wsu")
            nc.sync.dma_start(out=wsu_t[:], in_=wsu[:])
            wsd_t = sh_pool.tile([P, 4, 2048], BF16, tag="wsd")
            nc.sync.dma_start(out=wsd_t[:], in_=wsd[:])

            # ---- expert input loads: even experts on gpsimd queue, odd on
            # sync (after the shared tensors). Ring bufs gate the prefetch.
            xd_tiles, wu_tiles, wd_tiles = [], [], []
            for e in range(ELOC):
                q = nc.gpsimd if e % 2 == 0 else nc.sync
                S = caps[e]
                xd = xd_pool.tile([P, KC, S], BF16, tag="xd",
                                  padded_shape=[P, KC, Smax])
                q.dma_start(out=xd[:], in_=xdp[:, :, offs[e]:offs[e] + S])
                wu_t = wu_pool.tile([P, KC, 512], BF16, tag="wu")
                q.dma_start(out=wu_t[:], in_=wup[:, :, e * 512:(e + 1) * 512])
                wd_t = wd_pool.tile([P, 2, 2048], BF16, tag="wd")
                q.dma_start(out=wd_t[:], in_=wdn[:, :, e * 2048:(e + 1) * 2048])
                xd_tiles.append(xd)
                wu_tiles.append(wu_t)
                wd_tiles.append(wd_t)

            # ---- shared expert: SwiGLU, chunk order y first, gate second --
            # g half (output cols 512:1024 = m-chunks 4..7) first, silu to
            # SBUF; then y half, fused mul from PSUM.
            sg_sh = []
            for m in range(4, 8):
                pt = pu_pool.tile([P, NLOC], F32, space="PSUM", tag="pu")
                for k in range(KC):
                    nc.tensor.matmul(
                        out=pt[:],
                        lhsT=wsu_t[:, k, m * P:(m + 1) * P],
                        rhs=xs_t[:, k, :],
                        start=(k == 0), stop=(k == KC - 1))
                sg = sg_pool.tile([P, NLOC], F32, tag="sg")
                nc.scalar.activation(sg[:], pt[:],
                                     mybir.ActivationFunctionType.Silu)
                sg_sh.append(sg)
            hsh_tiles = []
            for m in range(4):
                pt = pu_pool.tile([P, NLOC], F32, space="PSUM", tag="pu")
                for k in range(KC):
                    nc.tensor.matmul(
                        out=pt[:],
                        lhsT=wsu_t[:, k, m * P:(m + 1) * P],
                        rhs=xs_t[:, k, :],
                        start=(k == 0), stop=(k == KC - 1))
                ht = h_pool.tile([P, NLOC], BF16, tag="h")
                nc.vector.tensor_mul(ht[:], sg_sh[m][:], pt[:])
                hsh_tiles.append(ht)

            # shared down-projection, transposed: out[c-chunk, token]
            for cc in range(KC):
                pt = pd_pool.tile([P, NLOC], F32, space="PSUM", tag="pd")
                for j in range(4):
                    nc.tensor.matmul(
                        out=pt[:],
                        lhsT=wsd_t[:, j, cc * P:(cc + 1) * P],
                        rhs=hsh_tiles[j][:],
                        start=(j == 0), stop=(j == 3))
                ot = o_pool.tile([P, NLOC], BF16, tag="o")
                nc.vector.tensor_copy(out=ot[:], in_=pt[:])
                nc.scalar.dma_start(out=yshT[cc * P:(cc + 1) * P, :], in_=ot[:])

            # ---- routed experts ------------------------------------------
            for e in range(ELOC):
                S = caps[e]
                xd, wu_t, wd_t = xd_tiles[e], wu_tiles[e], wd_tiles[e]
                for cs0 in range(0, S, 512):
                    W = min(512, S - cs0)
                    # up-projection: psum[m] = [128 of 2H, W slots]
                    up_ps = []
                    for m in range(4):
                        pt = pu_pool.tile([P, W], F32, space="PSUM", tag="pu",
                                          padded_shape=[P, 512])
                        for k in range(KC):
                            nc.tensor.matmul(
                                out=pt[:],
                                lhsT=wu_t[:, k, m * P:(m + 1) * P],
                                rhs=xd[:, k, cs0:cs0 + W],
                                start=(k == 0), stop=(k == KC - 1))
                        up_ps.append(pt)
                    # routed chunk order: gate first (m 0..1), up second (2..3)
                    h_tiles = []
                    for j in range(2):
                        sg = sg_pool.tile([P, W], F32, tag="sg",
                                          padded_shape=[P, 512])
                        nc.scalar.activation(sg[:], up_ps[j][:],
                                             mybir.ActivationFunctionType.Silu)
                        ht = h_pool.tile([P, W], BF16, tag="h",
                                         padded_shape=[P, 512])
                        nc.vector.tensor_mul(ht[:], sg[:], up_ps[2 + j][:])
                        h_tiles.append(ht)
                    # down-projection, transposed: out[c-chunk, slot]
                    for cc in range(KC):
                        pt = pd_pool.tile([P, W], F32, space="PSUM", tag="pd",
                                          padded_shape=[P, 512])
                        for j in range(2):
                            nc.tensor.matmul(
                                out=pt[:],
                                lhsT=wd_t[:, j, cc * P:(cc + 1) * P],
                                rhs=h_tiles[j][:],
                                start=(j == 0), stop=(j == 1))
                        ot = o_pool.tile([P, W], BF16, tag="o",
                                         padded_shape=[P, 512])
                        nc.vector.tensor_copy(out=ot[:], in_=pt[:])
                        c0 = offs[e] + cs0
                        nc.scalar.dma_start(
                            out=yrT[cc * P:(cc + 1) * P, c0:c0 + W], in_=ot[:])
    return nc


def _pack_pmajor(a, nchunk):
    """[nchunk*128, F] -> [128, nchunk, F] p-major packed, contiguous."""
    F = a.shape[1]
    return np.ascontiguousarray(
        a.reshape(nchunk, P, F).transpose(1, 0, 2))


# --------------------------------------------------------------------------
# Host wrapper
# --------------------------------------------------------------------------
def kernel(x, w_gate, w_shared_up, w_shared_down, w_up, w_down):
    x_flat = x.reshape(-1, C)

    # ---- gate: sigmoid scores, top-6, normalized weights (f64 for a stable
    # ordering; ties in the fp32 reference are measure-zero) ----
    logits = x_flat.astype(np.float64) @ w_gate.astype(np.float64)
    scores = 1.0 / (1.0 + np.exp(-logits))
    topk_idx = np.argsort(-scores, axis=-1, kind="stable")[:, :TOPK]
    w = np.take_along_axis(scores, topk_idx, axis=-1)
    w = w / w.sum(-1, keepdims=True)

    # ---- dispatch positions (stable within each expert, token-major) ----
    flat_e = topk_idx.reshape(-1)
    order = np.argsort(flat_e, kind="stable")
    sorted_e = flat_e[order]
    group_start = np.searchsorted(sorted_e, np.arange(E))
    pos = np.empty(N * TOPK, dtype=np.int64)
    pos[order] = np.arange(N * TOPK) - group_start[sorted_e]
    counts = np.bincount(flat_e, minlength=E)
    counts_c = np.minimum(counts, CAP)  # reference drops beyond CAP

    # ---- snake-deal experts to cores by descending count so all cores get
    # the same per-position capacity profile ----
    rank = np.argsort(-counts_c, kind="stable")  # expert ids, count desc
    assign = np.empty((NCORES, ELOC), dtype=int)  # [core, position] -> expert
    for j in range(ELOC):
        blk = rank[j * NCORES:(j + 1) * NCORES]
        assign[:, j] = blk if j % 2 == 0 else blk[::-1]
    core_of = np.empty(E, dtype=int)
    posn_of = np.empty(E, dtype=int)
    for c in range(NCORES):
        for j in range(ELOC):
            core_of[assign[c, j]] = c
            posn_of[assign[c, j]] = j
    caps = tuple(
        int(-(-max(8, counts_c[assign[:, j]].max()) // 8) * 8)
        for j in range(ELOC))
    S_tot = sum(caps)
    offs = np.concatenate([[0], np.cumsum(caps)]).astype(int)

    # ---- build per-core inputs ----
    xT_bf = np.ascontiguousarray(x_flat.T).astype(_BF16_NP)  # [C, N]
    wsu_p = _pack_pmajor(w_shared_up.astype(_BF16_NP), KC)
    wsd_p = _pack_pmajor(w_shared_down.astype(_BF16_NP), 4)

    token_of_slot = np.arange(N * TOPK) // TOPK
    expert_tokens = []
    for e in range(E):
        slots = order[group_start[e]: group_start[e] + counts_c[e]]
        expert_tokens.append(token_of_slot[slots])

    in_maps = []
    for c in range(NCORES):
        cols = np.zeros(S_tot, dtype=np.int64)
        for j in range(ELOC):
            tok = expert_tokens[assign[c, j]]
            cols[offs[j]:offs[j] + len(tok)] = tok
        gx = xT_bf[:, cols]                        # [C, S_tot]
        xdp = _pack_pmajor(gx, KC)                 # [128, 16, S_tot]
        xsp = _pack_pmajor(
            np.ascontiguousarray(xT_bf[:, c * NLOC:(c + 1) * NLOC]), KC)
        eids = assign[c]
        wupc = w_up[eids].astype(_BF16_NP)         # [8, 2048, 512]
        wup_p = np.ascontiguousarray(
            wupc.reshape(ELOC, KC, P, 512).transpose(2, 1, 0, 3)
        ).reshape(P, KC, ELOC * 512)
        wdnc = w_down[eids].astype(_BF16_NP)       # [8, 256, 2048]
        wdn_p = np.ascontiguousarray(
            wdnc.reshape(ELOC, 2, P, 2048).transpose(2, 1, 0, 3)
        ).reshape(P, 2, ELOC * 2048)
        in_maps.append({
            "xdp": xdp,
            "wup": wup_p,
            "wdn": wdn_p,
            "xsp": xsp,
            "wsu": wsu_p,
            "wsd": wsd_p,
        })

    if caps not in _BUILD_CACHE:
        _BUILD_CACHE[caps] = _build(caps)
    nc = _BUILD_CACHE[caps]

    res = run_bass_kernel_spmd(nc, in_maps, list(range(NCORES)))
    if res.exec_time_ns is not None:
        print(f"HW exec time: {res.exec_time_ns} ns", flush=True)

    # ---- host combine (unshard): gather per-slot cols, weight, sum ----
    yr_all = np.concatenate([r["yrT"] for r in res.results], axis=1)
    valid = pos < CAP
    pos_c = np.where(valid, pos, 0)
    gcol = (core_of[flat_e] * S_tot + offs[posn_of[flat_e]] + pos_c)
    y_ts = yr_all[:, gcol].T.astype(np.float32)    # [N*K, C]
    if not valid.all():
        y_ts[~valid] = 0.0
    routed = (y_ts.reshape(N, TOPK, C)
              * w.reshape(N, TOPK, 1).astype(np.float32)).sum(axis=1)
    shared = np.concatenate(
        [r["yshT"] for r in res.results], axis=1).T.astype(np.float32)
    return (shared + routed).reshape(B, T, C).astype(np.float32)


# revision 8
# speedup vs baseline: 1.1990x; 1.0070x over previous
"""MoE routing kernel for Trainium2, expert-parallel across 8 NeuronCores.

Sharding: experts are snake-dealt to cores by descending token count so every
core sees the same per-position capacity profile (single SPMD program).  The
gate/top-k/dispatch runs on host as part of the sharding step; each core
receives its experts' dispatched token columns packed contiguously (exact
counts rounded to 8 — no fixed-capacity padding), its expert weights, and a
token slice for the replicated shared expert.  The device computes the grouped
SwiGLU expert GEMMs (down-projection transposed so cost scales with the exact
slot count) plus the shared expert.  Host gathers per-slot outputs and does
the weighted combine.
"""

import numpy as np
import ml_dtypes

import bass_rust
import concourse.bass as bass
import concourse.mybir as mybir
from concourse.tile import TileContext
from concourse.vector_clock import ScopedClock
from concourse.bass_utils import run_bass_kernel_spmd

B, T, C = 2, 2048, 2048
N = B * T
E, H, HS = 64, 256, 512
TOPK = 6
CAP = 1024
NCORES = 8
ELOC = E // NCORES   # 8 experts per core
NLOC = N // NCORES   # 512 tokens per core for the shared expert
BF16 = mybir.dt.bfloat16
F32 = mybir.dt.float32
P = 128
KC = C // P          # 16 contraction chunks over C

_BF16_NP = ml_dtypes.bfloat16


# --------------------------------------------------------------------------
# Tile tail-drain fix: this walrus build allows at most one semaphore wait per
# instruction (none on Drain). Tile's end-of-context drain carries the whole
# global clock; emit a chain of single-wait NOPs on SP instead.
# --------------------------------------------------------------------------
def _patched_drain_and_barrier(self, tick_clock, wait_clock):
    carrier = self.nc.sync.nop(nofuse=True, hint="tail_wait_0")
    wait_clock.add_sem_waits(carrier.ins, ScopedClock({None: tick_clock.global_clock}))
    si = carrier.ins.sync_info
    waits = list(si.on_wait) if si else []
    upds = list(si.on_update) if si else []
    carrier.ins.sync_info = bass_rust.SyncInfo(on_wait=waits[:1], on_update=upds)
    for i, w in enumerate(waits[1:]):
        n2 = self.nc.sync.nop(nofuse=True, hint=f"tail_wait_{i + 1}")
        n2.ins.sync_info = bass_rust.SyncInfo(on_wait=[w], on_update=[])

    self.nc.sync.drain()
    self.nc.all_engine_barrier()
    assert self.sems is not None
    popped = self.nc._tile_sem_poison_stack.pop()
    assert popped is self._sem_poison
    self.nc.clear_and_free_semaphores(list(self.sems.allocated().values()))
    self.nc.all_engine_barrier()


_orig_add_instruction = TileContext._add_instruction


def _patched_add_instruction(self, inst):
    si = getattr(inst, "sync_info", None)
    if si is not None and len(si.on_wait) > 1:
        waits = list(si.on_wait)
        for w in waits[:-1]:
            nop = mybir.InstNoOp(
                name=self.nc.get_next_instruction_name(), ins=[], outs=[])
            nop.engine = inst.engine
            nop.sync_info = bass_rust.SyncInfo(on_wait=[w], on_update=[])
            _orig_add_instruction(self, nop)
        inst.sync_info = bass_rust.SyncInfo(
            on_wait=[waits[-1]], on_update=list(si.on_update))
    _orig_add_instruction(self, inst)


def _install_drain_fix():
    if getattr(TileContext, "_drain_fix_installed", False):
        return
    TileContext._drain_and_barrier = _patched_drain_and_barrier
    TileContext._add_instruction = _patched_add_instruction
    TileContext._drain_fix_installed = True


# --------------------------------------------------------------------------
# Device kernel
# --------------------------------------------------------------------------
_BUILD_CACHE = {}


def _build(caps):
    """Per-core Bass program; caps = per-position slot capacities (all cores
    share the profile, multiples of 8)."""
    _install_drain_fix()
    nc = bass.Bass()
    S_tot = sum(caps)
    Smax = max(caps)
    offs = np.concatenate([[0], np.cumsum(caps)]).astype(int)

    # p-major packed inputs: element [p, k, j] = full[k*128 + p, j]
    xdp = nc.declare_dram_parameter("xdp", [P, KC, S_tot], BF16, isOutput=False)
    wup = nc.declare_dram_parameter("wup", [P, KC, ELOC * 512], BF16, isOutput=False)
    wdn = nc.declare_dram_parameter("wdn", [P, 2, ELOC * 2048], BF16, isOutput=False)
    xsp = nc.declare_dram_parameter("xsp", [P, KC, NLOC], BF16, isOutput=False)
    wsu = nc.declare_dram_parameter("wsu", [P, KC, 1024], BF16, isOutput=False)
    wsd = nc.declare_dram_parameter("wsd", [P, 4, 2048], BF16, isOutput=False)
    # transposed outputs: [C row, slot/token col]
    yrT = nc.declare_dram_parameter("yrT", [C, S_tot], BF16, isOutput=True)
    yshT = nc.declare_dram_parameter("yshT", [C, NLOC], BF16, isOutput=True)

    with TileContext(nc) as tc:
        with (
            tc.tile_pool(name="sh_sb", bufs=1) as sh_pool,
            tc.tile_pool(name="xd_sb", bufs=3) as xd_pool,
            tc.tile_pool(name="wu_sb", bufs=2) as wu_pool,
            tc.tile_pool(name="wd_sb", bufs=3) as wd_pool,
            tc.tile_pool(name="sg_sb", bufs=4) as sg_pool,
            tc.tile_pool(name="h_sb", bufs=4) as h_pool,
            tc.tile_pool(name="o_sb", bufs=16) as o_pool,
            tc.tile_pool(name="pu", bufs=4, space="PSUM") as pu_pool,
            tc.tile_pool(name="pd", bufs=4, space="PSUM") as pd_pool,
        ):
            # ---- shared expert: SwiGLU, chunk order y first, gate second --
            # chunked xs/wsu loads so the first accumulation group streams
            # behind the DMA (PE starts after ~384KB instead of 6.3MB);
            # g half (m-chunks 4..7) first, then y half from resident chunks.
            xs_ch, wsu_ch = [], []
            for k in range(KC):
                xk = sh_pool.tile([P, NLOC], BF16, tag="xsc", bufs=KC,
                                  name=f"xsc{k}")
                nc.sync.dma_start(out=xk[:], in_=xsp[:, k, :])
                wk = sh_pool.tile([P, 1024], BF16, tag="wsuc", bufs=KC,
                                  name=f"wsuc{k}")
                nc.sync.dma_start(out=wk[:], in_=wsu[:, k, :])
                xs_ch.append(xk)
                wsu_ch.append(wk)
            g_ps, y_ps = [], []
            for m in (4, 5, 6, 7, 0, 1, 2, 3):
                pool, tag = (pu_pool, "pu") if m >= 4 else (pd_pool, "pd")
                pt = pool.tile([P, NLOC], F32, space="PSUM", tag=tag,
                               name=f"shps{m}")
                for k in range(KC):
                    nc.tensor.matmul(
                        out=pt[:],
                        lhsT=wsu_ch[k][:, m * P:(m + 1) * P],
                        rhs=xs_ch[k][:],
                        start=(k == 0), stop=(k == KC - 1))
                (g_ps if m >= 4 else y_ps).append(pt)
            wsd_t = sh_pool.tile([P, 4, 2048], BF16, tag="wsd")
            nc.sync.dma_start(out=wsd_t[:], in_=wsd[:])

            # ---- expert input loads: even experts on gpsimd queue (starts
            # at t=0), odd on sync after the shared tensors. Ring bufs gate
            # the prefetch depth.
            xd_tiles, wu_tiles, wd_tiles = [], [], []
            for e in range(ELOC):
                q = nc.gpsimd if e % 2 == 0 else nc.sync
                S = caps[e]
                xd = xd_pool.tile([P, KC, S], BF16, tag="xd",
                                  padded_shape=[P, KC, Smax])
                q.dma_start(out=xd[:], in_=xdp[:, :, offs[e]:offs[e] + S])
                wu_t = wu_pool.tile([P, KC, 512], BF16, tag="wu")
                q.dma_start(out=wu_t[:], in_=wup[:, :, e * 512:(e + 1) * 512])
                wd_t = wd_pool.tile([P, 2, 2048], BF16, tag="wd")
                q.dma_start(out=wd_t[:], in_=wdn[:, :, e * 2048:(e + 1) * 2048])
                xd_tiles.append(xd)
                wu_tiles.append(wu_t)
                wd_tiles.append(wd_t)

            sg_sh = []
            for m in range(4):
                sg = sg_pool.tile([P, NLOC], F32, tag="sg", name=f"sgsh{m}")
                nc.scalar.activation(sg[:], g_ps[m][:],
                                     mybir.ActivationFunctionType.Silu)
                sg_sh.append(sg)
            hsh_tiles = []
            for m in range(4):
                ht = h_pool.tile([P, NLOC], BF16, tag="hsh", name=f"hsh{m}")
                nc.vector.tensor_mul(ht[:], sg_sh[m][:], y_ps[m][:])
                hsh_tiles.append(ht)

            # shared down-projection, transposed: out[c-chunk, token]
            for cc in range(KC):
                pt = pd_pool.tile([P, NLOC], F32, space="PSUM", tag="pd")
                for j in range(4):
                    nc.tensor.matmul(
                        out=pt[:],
                        lhsT=wsd_t[:, j, cc * P:(cc + 1) * P],
                        rhs=hsh_tiles[j][:],
                        start=(j == 0), stop=(j == 3))
                ot = o_pool.tile([P, NLOC], BF16, tag="o")
                nc.vector.tensor_copy(out=ot[:], in_=pt[:])
                nc.scalar.dma_start(out=yshT[cc * P:(cc + 1) * P, :], in_=ot[:])

            # ---- routed experts ------------------------------------------
            for e in range(ELOC):
                S = caps[e]
                xd, wu_t, wd_t = xd_tiles[e], wu_tiles[e], wd_tiles[e]
                for cs0 in range(0, S, 512):
                    W = min(512, S - cs0)
                    # up-projection: psum[m] = [128 of 2H, W slots]
                    up_ps = []
                    for m in range(4):
                        pt = pu_pool.tile([P, W], F32, space="PSUM", tag="pu",
                                          padded_shape=[P, 512])
                        for k in range(KC):
                            nc.tensor.matmul(
                                out=pt[:],
                                lhsT=wu_t[:, k, m * P:(m + 1) * P],
                                rhs=xd[:, k, cs0:cs0 + W],
                                start=(k == 0), stop=(k == KC - 1))
                        up_ps.append(pt)
                    # routed chunk order: gate first (m 0..1), up second (2..3)
                    h_tiles = []
                    for j in range(2):
                        sg = sg_pool.tile([P, W], F32, tag="sg",
                                          padded_shape=[P, 512])
                        nc.scalar.activation(sg[:], up_ps[j][:],
                                             mybir.ActivationFunctionType.Silu)
                        ht = h_pool.tile([P, W], BF16, tag="h",
                                         padded_shape=[P, 512])
                        nc.vector.tensor_mul(ht[:], sg[:], up_ps[2 + j][:])
                        h_tiles.append(ht)
                    # down-projection, transposed: out[c-chunk, slot]
                    last = (e == ELOC - 1)
                    for cc in range(KC):
                        pt = pd_pool.tile([P, W], F32, space="PSUM", tag="pd",
                                          padded_shape=[P, 512])
                        for j in range(2):
                            nc.tensor.matmul(
                                out=pt[:],
                                lhsT=wd_t[:, j, cc * P:(cc + 1) * P],
                                rhs=h_tiles[j][:],
                                start=(j == 0), stop=(j == 1))
                        ot = o_pool.tile([P, W], BF16, tag="o",
                                         padded_shape=[P, 512])
                        # spread the tail drain of the final expert across
                        # both DMA queues (input issue is idle by then)
                        deng = nc.sync if (last and cc % 2) else nc.scalar
                        nc.vector.tensor_copy(out=ot[:], in_=pt[:])
                        c0 = offs[e] + cs0
                        deng.dma_start(
                            out=yrT[cc * P:(cc + 1) * P, c0:c0 + W], in_=ot[:])
    return nc


def _pack_pmajor(a, nchunk):
    """[nchunk*128, F] -> [128, nchunk, F] p-major packed, contiguous."""
    F = a.shape[1]
    return np.ascontiguousarray(
        a.reshape(nchunk, P, F).transpose(1, 0, 2))


# --------------------------------------------------------------------------
# Host wrapper
# --------------------------------------------------------------------------
def kernel(x, w_gate, w_shared_up, w_shared_down, w_up, w_down):
    x_flat = x.reshape(-1, C)

    # ---- gate: sigmoid scores, top-6, normalized weights (f64 for a stable
    # ordering; ties in the fp32 reference are measure-zero) ----
    logits = x_flat.astype(np.float64) @ w_gate.astype(np.float64)
    scores = 1.0 / (1.0 + np.exp(-logits))
    topk_idx = np.argsort(-scores, axis=-1, kind="stable")[:, :TOPK]
    w = np.take_along_axis(scores, topk_idx, axis=-1)
    w = w / w.sum(-1, keepdims=True)

    # ---- dispatch positions (stable within each expert, token-major) ----
    flat_e = topk_idx.reshape(-1)
    order = np.argsort(flat_e, kind="stable")
    sorted_e = flat_e[order]
    group_start = np.searchsorted(sorted_e, np.arange(E))
    pos = np.empty(N * TOPK, dtype=np.int64)
    pos[order] = np.arange(N * TOPK) - group_start[sorted_e]
    counts = np.bincount(flat_e, minlength=E)
    counts_c = np.minimum(counts, CAP)  # reference drops beyond CAP

    # ---- snake-deal experts to cores by descending count so all cores get
    # the same per-position capacity profile ----
    rank = np.argsort(-counts_c, kind="stable")  # expert ids, count desc
    assign = np.empty((NCORES, ELOC), dtype=int)  # [core, position] -> expert
    for j in range(ELOC):
        blk = rank[j * NCORES:(j + 1) * NCORES]
        assign[:, j] = blk if j % 2 == 0 else blk[::-1]
    core_of = np.empty(E, dtype=int)
    posn_of = np.empty(E, dtype=int)
    for c in range(NCORES):
        for j in range(ELOC):
            core_of[assign[c, j]] = c
            posn_of[assign[c, j]] = j
    caps = tuple(
        int(-(-max(8, counts_c[assign[:, j]].max()) // 8) * 8)
        for j in range(ELOC))
    S_tot = sum(caps)
    offs = np.concatenate([[0], np.cumsum(caps)]).astype(int)

    # ---- build per-core inputs ----
    xT_bf = np.ascontiguousarray(x_flat.T).astype(_BF16_NP)  # [C, N]
    wsu_p = _pack_pmajor(w_shared_up.astype(_BF16_NP), KC)
    wsd_p = _pack_pmajor(w_shared_down.astype(_BF16_NP), 4)

    token_of_slot = np.arange(N * TOPK) // TOPK
    expert_tokens = []
    for e in range(E):
        slots = order[group_start[e]: group_start[e] + counts_c[e]]
        expert_tokens.append(token_of_slot[slots])

    in_maps = []
    for c in range(NCORES):
        cols = np.zeros(S_tot, dtype=np.int64)
        for j in range(ELOC):
            tok = expert_tokens[assign[c, j]]
            cols[offs[j]:offs[j] + len(tok)] = tok
        gx = xT_bf[:, cols]                        # [C, S_tot]
        xdp = _pack_pmajor(gx, KC)                 # [128, 16, S_tot]
        xsp = _pack_pmajor(
            np.ascontiguousarray(xT_bf[:, c * NLOC:(c + 1) * NLOC]), KC)
        eids = assign[c]
        wupc = w_up[eids].astype(_BF16_NP)         # [8, 2048, 512]
        wup_p = np.ascontiguousarray(
            wupc.reshape(ELOC, KC, P, 512).transpose(2, 1, 0, 3)
        ).reshape(P, KC, ELOC * 512)
        wdnc = w_down[eids].astype(_BF16_NP)       # [8, 256, 2048]
        wdn_p = np.ascontiguousarray(
            wdnc.reshape(ELOC, 2, P, 2048).transpose(2, 1, 0, 3)
        ).reshape(P, 2, ELOC * 2048)
        in_maps.append({
            "xdp": xdp,
            "wup": wup_p,
            "wdn": wdn_p,
            "xsp": xsp,
            "wsu": wsu_p,
            "wsd": wsd_p,
        })

    if caps not in _BUILD_CACHE:
        _BUILD_CACHE[caps] = _build(caps)
    nc = _BUILD_CACHE[caps]

    res = run_bass_kernel_spmd(nc, in_maps, list(range(NCORES)))
    if res.exec_time_ns is not None:
        print(f"HW exec time: {res.exec_time_ns} ns", flush=True)

    # ---- host combine (unshard): gather per-slot cols, weight, sum ----
    yr_all = np.concatenate([r["yrT"] for r in res.results], axis=1)
    valid = pos < CAP
    pos_c = np.where(valid, pos, 0)
    gcol = (core_of[flat_e] * S_tot + offs[posn_of[flat_e]] + pos_c)
    y_ts = yr_all[:, gcol].T.astype(np.float32)    # [N*K, C]
    if not valid.all():
        y_ts[~valid] = 0.0
    routed = (y_ts.reshape(N, TOPK, C)
              * w.reshape(N, TOPK, 1).astype(np.float32)).sum(axis=1)
    shared = np.concatenate(
        [r["yshT"] for r in res.results], axis=1).T.astype(np.float32)
    return (shared + routed).reshape(B, T, C).astype(np.float32)
